# revision 1
# baseline (speedup 1.0000x reference)
"""Multi-head self-attention on 8 Trainium2 NeuronCores.

Problem: B=2, S=2048, D=1024, H=16 heads (DK=64), fp32.

Sharding (8 cores): core c handles batch b = c//4 and head group g = c%4
(4 heads = 256 of the 1024 projection dims).  QKV are column-parallel,
Wo is row-parallel; the 4 partial outputs per batch are summed on the
host (cheap numpy add) together with a folded constant bias vector.

Device kernel (per core, identical SPMD program), bf16 matmul operands:
  - inputs are pre-transposed and pre-cast to bf16 on host (no on-device
    transposes): xT [1024,2048], wqT/wkT/wvT [1024,256], woT [256,1024].
  - V is projected for all 4 local heads up front; Q^T/K^T are projected
    PER HEAD, software-pipelined as TensorE filler inside the previous
    head's attention loop.  This keeps the PE continuously busy while
    ScalarE runs the exps, so the HAM clock gate stays at 2.4 GHz
    (an ACT-bound attention loop lets the PE micro-idle, HAM rethrottles
    to 1.2 GHz, and every matmul doubles in cost — measured 485us).
  - scores^T layout [kk, q] per (head, q-half): matmul -> PSUM[128,1024],
    exp(s/8 + mask_bias) fused on ScalarE -> bf16 P^T tiles,
    P^T @ V' (ones-column appended to V) accumulates context^T and the
    softmax denominators in one PSUM tile.
  - context is evicted unnormalized; the denominator row is reshaped to
    [128, 16] via a DRAM bounce so the iterative-divide reciprocal runs
    on 128 lanes (a [1, 2048] reciprocal costs ~13us), then broadcast
    back along partitions by DMA and applied with one tensor multiply.
  - Wo projection of normalized context^T -> partial out [2048, 1024].

PSUM budget (8 banks): score tiles [128,1024] x2 bufs (4) + context
accumulator [128,1024] (2) + projection tiles [128,512] x2 bufs (2).

Math notes (exactness):
  - K bias cancels in softmax (adds a per-query constant to scores).
  - V bias commutes: softmax(S) @ (V + 1 b_v^T) = softmax(S) @ V + b_v^T,
    so it is added on the host as Wo_w @ Wv_b (+ Wo_b) once per batch.
"""

import sys

for _p in ("/root/.axon_site", "/root/.axon_site/_ro/trn_rl_repo",
           "/root/.axon_site/_ro/pypackages", "/opt/trn_rl_repo"):
    if _p not in sys.path:
        sys.path.append(_p)

import ml_dtypes
import numpy as np

import concourse.bass as bass
import concourse.tile as tile
from concourse import bacc, mybir
from concourse.bass_utils import run_bass_kernel_spmd

B, S, D, H = 2, 2048, 1024, 16
DK = D // H          # 64 head dim
NCORES = 8
HL = H // 4          # 4 heads per core
CL = HL * DK         # 256 local context dims per core
P = 128
EC = D // P          # 8 contraction chunks
F32 = mybir.dt.float32
BF16 = mybir.dt.bfloat16
AF = mybir.ActivationFunctionType
BF = ml_dtypes.bfloat16

KT_TILES = S // P    # 16 key tiles
QW = 512             # matmul moving-dim chunk
SCW = 1024           # score-tile q width (one PSUM score tile)
NQH = S // SCW       # 2 q-halves per head

LAST_RESULT = None   # BassKernelResults of the most recent run (for test.py)


def build_program():
    nc = bacc.Bacc("TRN2", target_bir_lowering=False, debug=False,
                   num_devices=NCORES)
    xT = nc.dram_tensor("xT", [D, S], BF16, kind="ExternalInput")
    wqT = nc.dram_tensor("wqT", [D, CL], BF16, kind="ExternalInput")
    wkT = nc.dram_tensor("wkT", [D, CL], BF16, kind="ExternalInput")
    wvT = nc.dram_tensor("wvT", [D, CL], BF16, kind="ExternalInput")
    bq4 = nc.dram_tensor("bq4", [DK, HL], F32, kind="ExternalInput")
    mb = nc.dram_tensor("mb", [P, KT_TILES], F32, kind="ExternalInput")
    woT = nc.dram_tensor("woT", [CL, D], BF16, kind="ExternalInput")
    pout = nc.dram_tensor("pout", [S, D], F32, kind="ExternalOutput")

    with tile.TileContext(nc) as tc:
        with (
            tc.tile_pool(name="consts", bufs=1) as consts,
            tc.tile_pool(name="work", bufs=1) as work,
            tc.tile_pool(name="psum", bufs=1, space="PSUM") as psum,
            tc.tile_pool(name="dramp", bufs=2, space="DRAM") as dramp,
        ):
            # persistent SBUF tensors
            xt_sb = consts.tile([P, EC, S], BF16)
            wq_sb = consts.tile([P, EC, CL], BF16)
            wk_sb = consts.tile([P, EC, CL], BF16)
            wv_sb = consts.tile([P, EC, CL], BF16)
            v_sb = consts.tile([P, KT_TILES, HL, DK + 1], BF16)  # V + ones col
            ctxn = consts.tile([P, 2, S], BF16)                  # normalized ctx^T
            bq_sb = consts.tile([DK, HL], F32)
            mb_sb = consts.tile([P, KT_TILES], F32)
            wo_sb = consts.tile([P, 2, D], BF16)

            # load order matters: the V projection (first PE work) needs wv
            # and xt chunk e as its e-loop reaches it, so those go first.
            nc.sync.dma_start(out=wv_sb, in_=wvT.rearrange("(j p) c -> p j c", p=P))
            xr = xT.rearrange("(j p) q -> p j q", p=P)
            nc.sync.dma_start(out=xt_sb[:, 0, :], in_=xr[:, 0, :])
            nc.sync.dma_start(out=xt_sb[:, 1, :], in_=xr[:, 1, :])
            nc.sync.dma_start(out=wq_sb, in_=wqT.rearrange("(j p) c -> p j c", p=P))
            for e in range(2, EC):
                nc.sync.dma_start(out=xt_sb[:, e, :], in_=xr[:, e, :])
            nc.sync.dma_start(out=wk_sb, in_=wkT.rearrange("(j p) c -> p j c", p=P))
            nc.sync.dma_start(out=bq_sb, in_=bq4[:, :])
            nc.sync.dma_start(out=mb_sb, in_=mb[:, :])
            nc.sync.dma_start(out=wo_sb, in_=woT.rearrange("(j p) c -> p j c", p=P))
            nc.vector.memset(v_sb[:, :, :, DK:DK + 1], 1.0)

            # ---- V projection, all local heads up front ----
            for kt in range(KT_TILES):
                ps = psum.tile([P, QW], F32, tag="pj", bufs=2, name=f"pv{kt}")
                for e in range(EC):
                    nc.tensor.matmul(
                        ps[:, 0:CL],
                        lhsT=xt_sb[:, e, kt * P:(kt + 1) * P],
                        rhs=wv_sb[:, e, :],
                        start=(e == 0), stop=(e == EC - 1))
                nc.vector.tensor_copy(
                    out=v_sb[:, kt, :, 0:DK],
                    in_=ps[:, 0:CL].rearrange("p (h d) -> p h d", h=HL))

            # Per-head Q^T/K^T projection emitters.  Each returns a list of
            # closures (one matmul group each) so the caller can interleave
            # them as TensorE filler inside the previous head's attention.
            def qk_groups(h, qt_t, kt_t):
                groups = []
                for w_sb, o_t, is_q in ((wq_sb, qt_t, True), (wk_sb, kt_t, False)):
                    for qc in range(S // QW):
                        def g(w_sb=w_sb, o_t=o_t, is_q=is_q, qc=qc, h=h):
                            ps = psum.tile([P, QW], F32, tag="pj", bufs=2,
                                           name=f"pqk{h}_{int(is_q)}_{qc}")
                            for e in range(EC):
                                nc.tensor.matmul(
                                    ps[0:DK, :],
                                    lhsT=w_sb[:, e, h * DK:(h + 1) * DK],
                                    rhs=xt_sb[:, e, qc * QW:(qc + 1) * QW],
                                    start=(e == 0), stop=(e == EC - 1))
                            dst = o_t[:, qc * QW:(qc + 1) * QW]
                            if is_q:
                                nc.vector.tensor_scalar_add(
                                    out=dst, in0=ps[0:DK, :],
                                    scalar1=bq_sb[:, h:h + 1])
                            else:
                                nc.vector.tensor_copy(out=dst, in_=ps[0:DK, :])
                        groups.append(g)
                return groups

            def alloc_qk(h):
                qt_t = work.tile([DK, S], BF16, tag="qt", bufs=2, name=f"qt{h}")
                kt_t = work.tile([DK, S], BF16, tag="kt", bufs=2, name=f"kt{h}")
                return qt_t, kt_t

            # head 0's projections run up front
            cur_qk = alloc_qk(0)
            for g in qk_groups(0, *cur_qk):
                g()

            scale = 1.0 / float(np.sqrt(DK))
            for h in range(HL):
                qt_t, kt_t = cur_qk
                if h + 1 < HL:
                    nxt_qk = alloc_qk(h + 1)
                    filler = qk_groups(h + 1, *nxt_qk)
                else:
                    nxt_qk, filler = None, []
                fi = 0

                hb, hr = h // 2, (h % 2) * DK
                it = 0
                for qh in range(NQH):
                    q0 = qh * SCW
                    ctx_ps = psum.tile([P, SCW], F32, tag="ctx", bufs=1,
                                       name=f"ctx{h}_{qh}")
                    for kt in range(KT_TILES):
                        sc_ps = psum.tile([P, SCW], F32, tag="sc", bufs=2,
                                          name=f"sc{h}_{qh}_{kt}")
                        for c in range(SCW // QW):
                            nc.tensor.matmul(
                                sc_ps[:, c * QW:(c + 1) * QW],
                                lhsT=kt_t[:, kt * P:(kt + 1) * P],
                                rhs=qt_t[:, q0 + c * QW:q0 + (c + 1) * QW],
                                start=True, stop=True)
                        pt = work.tile([P, SCW], BF16, tag="pt", bufs=3,
                                       name=f"pt{h}_{qh}_{kt}")
                        nc.scalar.activation(out=pt, in_=sc_ps, func=AF.Exp,
                                             bias=mb_sb[:, kt:kt + 1],
                                             scale=scale)
                        for c in range(SCW // QW):
                            nc.tensor.matmul(
                                ctx_ps[0:DK + 1, c * QW:(c + 1) * QW],
                                lhsT=v_sb[:, kt, h, :],
                                rhs=pt[:, c * QW:(c + 1) * QW],
                                start=(kt == 0), stop=(kt == KT_TILES - 1))
                        # TensorE filler: next head's Q/K projection groups
                        if it % 4 == 1 and fi < len(filler):
                            filler[fi]()
                            fi += 1
                        it += 1
                    # evict unnormalized context + denominator row, then
                    # normalize this q-half (the chain overlaps the next
                    # q-half / head attention; DMA bounce reshapes the
                    # denominator so the reciprocal runs on 128 lanes)
                    ctxu = work.tile([DK, SCW], F32, tag="ctxu", bufs=3,
                                     name=f"ctxu{h}_{qh}")
                    nc.vector.tensor_copy(out=ctxu, in_=ctx_ps[0:DK, :])
                    den = work.tile([1, SCW], F32, tag="den", bufs=3,
                                    name=f"den{h}_{qh}")
                    nc.vector.tensor_copy(out=den, in_=ctx_ps[DK:DK + 1, :])
                    dd = dramp.tile([1, SCW], F32, tag="dd", name=f"dd{h}{qh}")
                    nc.sync.dma_start(out=dd, in_=den)
                    den2 = work.tile([P, SCW // P], F32, tag="den2", bufs=3,
                                     name=f"den2{h}_{qh}")
                    nc.sync.dma_start(
                        out=den2, in_=dd.rearrange("o (p f) -> (o p) f", p=P))
                    den2r = work.tile([P, SCW // P], F32, tag="den2r", bufs=3,
                                      name=f"den2r{h}_{qh}")
                    nc.vector.reciprocal(out=den2r, in_=den2)
                    dr = dramp.tile([1, SCW], F32, tag="dr", name=f"dr{h}{qh}")
                    nc.sync.dma_start(
                        out=dr.rearrange("o (p f) -> (o p) f", p=P), in_=den2r)
                    rb = work.tile([DK, SCW], F32, tag="rb", bufs=3,
                                   name=f"rb{h}_{qh}")
                    nc.sync.dma_start(out=rb, in_=dr.to_broadcast([DK, SCW]))
                    nc.vector.tensor_mul(out=ctxn[hr:hr + DK, hb, q0:q0 + SCW],
                                         in0=ctxu, in1=rb)
                while fi < len(filler):
                    filler[fi]()
                    fi += 1
                cur_qk = nxt_qk

            # ---- output projection ----
            for t in range(S // P):
                po = work.tile([P, D], F32, tag="po", bufs=3, name=f"po{t}")
                for dc in range(2):
                    ps = psum.tile([P, QW], F32, tag="pj", bufs=2,
                                   name=f"pw{t}_{dc}")
                    for cb in range(2):
                        nc.tensor.matmul(
                            ps,
                            lhsT=ctxn[:, cb, t * P:(t + 1) * P],
                            rhs=wo_sb[:, cb, dc * QW:(dc + 1) * QW],
                            start=(cb == 0), stop=(cb == 1))
                    nc.vector.tensor_copy(out=po[:, dc * QW:(dc + 1) * QW], in_=ps)
                nc.sync.dma_start(out=pout[t * P:(t + 1) * P, :], in_=po)

    nc.compile()
    return nc


_PROGRAM = None


def _get_program():
    global _PROGRAM
    if _PROGRAM is None:
        _PROGRAM = build_program()
    return _PROGRAM


def _bf(a):
    return np.ascontiguousarray(np.asarray(a, np.float32)).astype(BF)


def kernel(x, mask, Wq_w, Wq_b, Wk_w, Wk_b, Wv_w, Wv_b, Wo_w, Wo_b,
           **run_kwargs):
    global LAST_RESULT
    x = np.asarray(x, np.float32)
    mask = np.asarray(mask)
    Wq_w = np.asarray(Wq_w, np.float32)
    Wk_w = np.asarray(Wk_w, np.float32)
    Wv_w = np.asarray(Wv_w, np.float32)
    Wo_w = np.asarray(Wo_w, np.float32)

    nc = _get_program()

    xTs = [_bf(x[b].T) for b in range(B)]
    mbs = []
    for b in range(B):
        mrow = np.asarray(mask[b, 0, 0, :])
        bias = np.where(mrow == 0, np.float32(-50.0), np.float32(0.0))
        mbs.append(np.ascontiguousarray(bias.reshape(S // P, P).T.astype(np.float32)))

    in_maps = []
    for c in range(NCORES):
        b, g = c // 4, c % 4
        sl = slice(g * CL, (g + 1) * CL)
        in_maps.append({
            "xT": xTs[b],
            "wqT": _bf(Wq_w[sl, :].T),
            "wkT": _bf(Wk_w[sl, :].T),
            "wvT": _bf(Wv_w[sl, :].T),
            "bq4": np.ascontiguousarray(
                np.asarray(Wq_b, np.float32)[sl].reshape(HL, DK).T),
            "mb": mbs[b],
            "woT": _bf(Wo_w[:, sl].T),
        })

    res = run_bass_kernel_spmd(nc, in_maps, core_ids=list(range(NCORES)),
                               **run_kwargs)
    LAST_RESULT = res

    # host-side unshard: sum the 4 row-parallel partials per batch and add
    # the folded constant bias (Wo @ Wv_b + Wo_b).
    obias = (Wo_w @ np.asarray(Wv_b, np.float32)
             + np.asarray(Wo_b, np.float32)).astype(np.float32)
    out = np.empty((B, S, D), np.float32)
    for b in range(B):
        acc = res.results[4 * b]["pout"].astype(np.float32)
        for g in range(1, 4):
            acc = acc + res.results[4 * b + g]["pout"]
        out[b] = acc + obias
    return out



# revision 3
# speedup vs baseline: 1.3396x; 1.3396x over previous
"""Multi-head self-attention on 8 Trainium2 NeuronCores.

Problem: B=2, S=2048, D=1024, H=16 heads (DK=64), fp32.

Sharding (8 cores): core c handles batch b = c//4 and head group g = c%4
(4 heads = 256 of the 1024 projection dims).  QKV are column-parallel,
Wo is row-parallel; the 4 partial outputs per batch are summed on the
host (cheap numpy add) together with a folded constant bias vector.

Device schedule (per core, identical SPMD program), bf16 operands:
  - The attention inner loop is jointly limited by ScalarE (exp of a
    [128,1024] score tile = ~1.34us) and the PE (scores + ctx + filler
    matmuls = ~1.1us/kt).  The emission order per key tile kt is
    scores(kt) -> exp(kt) -> ctx(kt-1) -> filler, so the in-order PE
    queue never blocks on the exp semaphore: exp(kt) input is complete
    one full iteration before ctx(kt) consumes it.
  - Q/K projections are packed per head PAIR (stationary M=128 instead
    of 64): qt2/kt2 hold two heads stacked on partitions [0:64]/[64:128]
    and the score matmuls slice a 64-partition base offset (PE tiling
    position (64,0) for odd heads).  Halves the projection instruction
    count and PE cycles vs per-head M=64.
  - Projection work (next pair's Q/K, first-half Wo) is drained from a
    unit queue 1-2 matmuls per kt iteration as TensorE filler inside the
    ACT-bound attention loop.
  - x is DMA'd in 4 column groups so the V projection (which reads all
    8 contraction chunks but only kt's 128 columns) starts at ~25% fill.
  - scores^T layout [kk, q] per (head, q-half): matmul -> PSUM[128,1024],
    exp(s/8 + mask_bias) fused on ScalarE -> bf16 P^T tiles,
    P^T @ V' (ones-column appended to V) accumulates context^T and the
    softmax denominators in one PSUM tile.
  - context is evicted unnormalized; the denominator row is reshaped to
    [128, 16] via a DRAM bounce so the iterative-divide reciprocal runs
    on 128 lanes, then broadcast back along partitions by DMA and
    applied with one tensor multiply.
  - Wo projection of normalized context^T -> partial out [2048, 1024]
    in bf16 (halves the output DMA; host accumulates in fp32).

PSUM budget (8 banks): score tiles [128,1024] x2 bufs (4) + context
accumulator [128,1024] (2) + projection tiles [128,512] x2 bufs (2).

Math notes (exactness):
  - K bias cancels in softmax (adds a per-query constant to scores).
  - V bias commutes: softmax(S) @ (V + 1 b_v^T) = softmax(S) @ V + b_v^T,
    so it is added on the host as Wo_w @ Wv_b (+ Wo_b) once per batch.
"""

import sys

for _p in ("/root/.axon_site", "/root/.axon_site/_ro/trn_rl_repo",
           "/root/.axon_site/_ro/pypackages", "/opt/trn_rl_repo"):
    if _p not in sys.path:
        sys.path.append(_p)

import ml_dtypes
import numpy as np

import concourse.bass as bass
import concourse.tile as tile
from concourse import bacc, mybir
from concourse.bass_utils import run_bass_kernel_spmd

B, S, D, H = 2, 2048, 1024, 16
DK = D // H          # 64 head dim
NCORES = 8
HL = H // 4          # 4 heads per core
NP = HL // 2         # 2 head pairs per core
CL = HL * DK         # 256 local context dims per core
P = 128
EC = D // P          # 8 contraction chunks
F32 = mybir.dt.float32
BF16 = mybir.dt.bfloat16
AF = mybir.ActivationFunctionType
BF = ml_dtypes.bfloat16

KT_TILES = S // P    # 16 key tiles
QW = 512             # matmul moving-dim chunk
SCW = 1024           # score-tile q width (one PSUM score tile)
NQH = S // SCW       # 2 q-halves per head
XG = 4               # x DMA column groups

LAST_RESULT = None   # BassKernelResults of the most recent run (for test.py)


def build_program():
    nc = bacc.Bacc("TRN2", target_bir_lowering=False, debug=False,
                   num_devices=NCORES)
    xT = nc.dram_tensor("xT", [D, S], BF16, kind="ExternalInput")
    wqT = nc.dram_tensor("wqT", [D, CL], BF16, kind="ExternalInput")
    wkT = nc.dram_tensor("wkT", [D, CL], BF16, kind="ExternalInput")
    wvT = nc.dram_tensor("wvT", [D, CL], BF16, kind="ExternalInput")
    bqp = nc.dram_tensor("bqp", [P, NP], F32, kind="ExternalInput")
    mb = nc.dram_tensor("mb", [P, KT_TILES], F32, kind="ExternalInput")
    woT = nc.dram_tensor("woT", [CL, D], BF16, kind="ExternalInput")
    pout = nc.dram_tensor("pout", [S, D], BF16, kind="ExternalOutput")

    with tile.TileContext(nc) as tc:
        with (
            tc.tile_pool(name="consts", bufs=1) as consts,
            tc.tile_pool(name="work", bufs=1) as work,
            tc.tile_pool(name="psum", bufs=1, space="PSUM") as psum,
            tc.tile_pool(name="dramp", bufs=2, space="DRAM") as dramp,
        ):
            # persistent SBUF tensors
            xt_sb = consts.tile([P, EC, S], BF16)
            wq_sb = consts.tile([P, EC, CL], BF16)
            wk_sb = consts.tile([P, EC, CL], BF16)
            wv_sb = consts.tile([P, EC, CL], BF16)
            v_sb = consts.tile([P, KT_TILES, HL, DK + 1], BF16)  # V + ones col
            ctxn = consts.tile([P, 2, S], BF16)                  # normalized ctx^T
            bq_sb = consts.tile([P, NP], F32)
            mb_sb = consts.tile([P, KT_TILES], F32)
            wo_sb = consts.tile([P, 2, D], BF16)

            # DMA order: V projection consumes wv + x column group g for key
            # tiles 4g..4g+3, so wv and group 0 go first; weights for the
            # projections that follow stream in between the x groups.
            xr = xT.rearrange("(j p) q -> p j q", p=P)
            GW = S // XG
            nc.sync.dma_start(out=wv_sb, in_=wvT.rearrange("(j p) c -> p j c", p=P))
            for g in range(XG):
                for e in range(EC):
                    nc.sync.dma_start(
                        out=xt_sb[:, e, g * GW:(g + 1) * GW],
                        in_=xr[:, e, g * GW:(g + 1) * GW])
                if g == 0:
                    nc.sync.dma_start(
                        out=wk_sb, in_=wkT.rearrange("(j p) c -> p j c", p=P))
                elif g == 1:
                    nc.sync.dma_start(
                        out=wq_sb, in_=wqT.rearrange("(j p) c -> p j c", p=P))
            nc.sync.dma_start(out=bq_sb, in_=bqp[:, :])
            nc.sync.dma_start(out=mb_sb, in_=mb[:, :])
            nc.sync.dma_start(out=wo_sb, in_=woT.rearrange("(j p) c -> p j c", p=P))
            nc.vector.memset(v_sb[:, :, :, DK:DK + 1], 1.0)

            # ---- V projection, all local heads up front ----
            for kt in range(KT_TILES):
                ps = psum.tile([P, QW], F32, tag="pj", bufs=2, name=f"pv{kt}")
                for e in range(EC):
                    nc.tensor.matmul(
                        ps[:, 0:CL],
                        lhsT=xt_sb[:, e, kt * P:(kt + 1) * P],
                        rhs=wv_sb[:, e, :],
                        start=(e == 0), stop=(e == EC - 1))
                nc.vector.tensor_copy(
                    out=v_sb[:, kt, :, 0:DK],
                    in_=ps[:, 0:CL].rearrange("p (h d) -> p h d", h=HL))

            # ---- Q/K projection unit emitters (packed per head pair) ----
            # Each unit = one accumulation matmul; the 8th unit of a group
            # also evicts the PSUM tile into qt2/kt2.  Units are drained
            # 1-2 per attention iteration as TensorE filler.
            def qk_units(pair, qt2, kt2, qcs_q, qcs_k):
                units = []
                plo, phi = pair * P, (pair + 1) * P

                def emit(w_sb, o_t, is_q, qc):
                    ps_box = {}

                    def unit(e, w_sb=w_sb, o_t=o_t, is_q=is_q, qc=qc):
                        if e == 0:
                            ps_box[0] = psum.tile(
                                [P, QW], F32, tag="pj", bufs=2,
                                name=f"pqk{pair}_{int(is_q)}_{qc}")
                        ps = ps_box[0]
                        nc.tensor.matmul(
                            ps,
                            lhsT=w_sb[:, e, plo:phi],
                            rhs=xt_sb[:, e, qc * QW:(qc + 1) * QW],
                            start=(e == 0), stop=(e == EC - 1))
                        if e == EC - 1:
                            dst = o_t[:, qc * QW:(qc + 1) * QW]
                            if is_q:
                                nc.vector.tensor_scalar_add(
                                    out=dst, in0=ps,
                                    scalar1=bq_sb[:, pair:pair + 1])
                            else:
                                nc.vector.tensor_copy(out=dst, in_=ps)
                    return [lambda e=e: unit(e) for e in range(EC)]

                for qc in qcs_k:
                    units.extend(emit(wk_sb, kt2, False, qc))
                for qc in qcs_q:
                    units.extend(emit(wq_sb, qt2, True, qc))
                return units

            def alloc_qk(pair):
                qt2 = work.tile([P, S], BF16, tag="qt", bufs=2, name=f"qt{pair}")
                kt2 = work.tile([P, S], BF16, tag="kt", bufs=2, name=f"kt{pair}")
                return qt2, kt2

            # pair 0: K (all) + Q (first half) up front; Q qc 2,3 become the
            # first filler units in head 0's attention.
            cur_qk = alloc_qk(0)
            for u in qk_units(0, *cur_qk, qcs_q=(0, 1), qcs_k=(0, 1, 2, 3)):
                u()
            filler = list(qk_units(0, *cur_qk, qcs_q=(2, 3), qcs_k=()))

            # ---- Wo unit emitters (4 matmuls + eviction + DMA per tile) ----
            po_ctr = [0]

            def wo_units(t0, t1, act_evict):
                units = []
                for t in range(t0, t1):
                    box = {}

                    def unit(step, t=t, box=box):
                        if step == 0:
                            box["po"] = work.tile([P, D], BF16, tag="po",
                                                  bufs=3, name=f"po{t}")
                        dc, cb = divmod(step, 2)
                        ps_name = f"pw{t}_{dc}"
                        if cb == 0:
                            box[dc] = psum.tile([P, QW], F32, tag="pj",
                                                bufs=2, name=ps_name)
                        nc.tensor.matmul(
                            box[dc],
                            lhsT=ctxn[:, cb, t * P:(t + 1) * P],
                            rhs=wo_sb[:, cb, dc * QW:(dc + 1) * QW],
                            start=(cb == 0), stop=(cb == 1))
                        if cb == 1:
                            dst = box["po"][:, dc * QW:(dc + 1) * QW]
                            if act_evict and po_ctr[0] % 2 == 0:
                                nc.scalar.copy(out=dst, in_=box[dc])
                            else:
                                nc.vector.tensor_copy(out=dst, in_=box[dc])
                            po_ctr[0] += 1
                            if dc == 1:
                                nc.sync.dma_start(
                                    out=pout[t * P:(t + 1) * P, :],
                                    in_=box["po"])
                    units.extend([lambda s=s, u=unit: u(s) for s in range(4)])
                return units

            scale = 1.0 / float(np.sqrt(DK))
            for h in range(HL):
                pair, hh = divmod(h, 2)
                off = hh * DK
                qt2, kt2 = cur_qk
                if h == 1:
                    # next pair's projections become filler for the rest of
                    # pair 0's attention
                    nxt_qk = alloc_qk(1)
                    filler.extend(qk_units(1, *nxt_qk, qcs_q=(0, 1, 2, 3),
                                           qcs_k=(0, 1, 2, 3)))
                hb, hr = h // 2, (h % 2) * DK
                for qh in range(NQH):
                    q0 = qh * SCW
                    last = (h == HL - 1) and (qh == NQH - 1)
                    ctx_ps = psum.tile([P, SCW], F32, tag="ctx", bufs=1,
                                       name=f"ctx{h}_{qh}")
                    prev_pt = None
                    for kt in range(KT_TILES):
                        sc_ps = psum.tile([P, SCW], F32, tag="sc", bufs=2,
                                          name=f"sc{h}_{qh}_{kt}")
                        for c in range(SCW // QW):
                            nc.tensor.matmul(
                                sc_ps[:, c * QW:(c + 1) * QW],
                                lhsT=kt2[off:off + DK, kt * P:(kt + 1) * P],
                                rhs=qt2[off:off + DK,
                                        q0 + c * QW:q0 + (c + 1) * QW],
                                start=True, stop=True)
                        pt = work.tile([P, SCW], BF16, tag="pt", bufs=3,
                                       name=f"pt{h}_{qh}_{kt}")
                        nc.scalar.activation(out=pt, in_=sc_ps, func=AF.Exp,
                                             bias=mb_sb[:, kt:kt + 1],
                                             scale=scale)
                        if prev_pt is not None:
                            pkt, ppt = prev_pt
                            for c in range(SCW // QW):
                                nc.tensor.matmul(
                                    ctx_ps[0:DK + 1, c * QW:(c + 1) * QW],
                                    lhsT=v_sb[:, pkt, h, :],
                                    rhs=ppt[:, c * QW:(c + 1) * QW],
                                    start=(pkt == 0), stop=False)
                        prev_pt = (kt, pt)
                        # TensorE filler: drain 1-2 projection units
                        nticks = 2 if kt < 8 else 1
                        if last and kt == 4:
                            # first-half Wo becomes filler once head 3's
                            # first-half context is normalized
                            filler.extend(wo_units(0, S // P // 2, False))
                        for _ in range(nticks):
                            if filler:
                                filler.pop(0)()
                    pkt, ppt = prev_pt
                    for c in range(SCW // QW):
                        nc.tensor.matmul(
                            ctx_ps[0:DK + 1, c * QW:(c + 1) * QW],
                            lhsT=v_sb[:, pkt, h, :],
                            rhs=ppt[:, c * QW:(c + 1) * QW],
                            start=False, stop=True)
                    # evict unnormalized context + denominator row, then
                    # normalize this q-half (the chain overlaps the next
                    # q-half / head attention; DMA bounce reshapes the
                    # denominator so the reciprocal runs on 128 lanes)
                    ctxu = work.tile([DK, SCW], F32, tag="ctxu", bufs=3,
                                     name=f"ctxu{h}_{qh}")
                    nc.vector.tensor_copy(out=ctxu, in_=ctx_ps[0:DK, :])
                    den = work.tile([1, SCW], F32, tag="den", bufs=3,
                                    name=f"den{h}_{qh}")
                    nc.vector.tensor_copy(out=den, in_=ctx_ps[DK:DK + 1, :])
                    dd = dramp.tile([1, SCW], F32, tag="dd", name=f"dd{h}{qh}")
                    nc.sync.dma_start(out=dd, in_=den)
                    den2 = work.tile([P, SCW // P], F32, tag="den2", bufs=3,
                                     name=f"den2{h}_{qh}")
                    nc.sync.dma_start(
                        out=den2, in_=dd.rearrange("o (p f) -> (o p) f", p=P))
                    den2r = work.tile([P, SCW // P], F32, tag="den2r", bufs=3,
                                      name=f"den2r{h}_{qh}")
                    nc.vector.reciprocal(out=den2r, in_=den2)
                    dr = dramp.tile([1, SCW], F32, tag="dr", name=f"dr{h}{qh}")
                    nc.sync.dma_start(
                        out=dr.rearrange("o (p f) -> (o p) f", p=P), in_=den2r)
                    rb = work.tile([DK, SCW], F32, tag="rb", bufs=3,
                                   name=f"rb{h}_{qh}")
                    nc.sync.dma_start(out=rb, in_=dr.to_broadcast([DK, SCW]))
                    nc.vector.tensor_mul(out=ctxn[hr:hr + DK, hb, q0:q0 + SCW],
                                         in0=ctxu, in1=rb)
                if pair == 0 and hh == 1:
                    cur_qk = nxt_qk

            # ---- remaining output projection (second q-half + leftovers) ----
            for u in filler:
                u()
            for u in wo_units(S // P // 2, S // P, True):
                u()

    nc.compile()
    return nc


_PROGRAM = None


def _get_program():
    global _PROGRAM
    if _PROGRAM is None:
        _PROGRAM = build_program()
    return _PROGRAM


def _bf(a):
    return np.ascontiguousarray(np.asarray(a, np.float32)).astype(BF)


def kernel(x, mask, Wq_w, Wq_b, Wk_w, Wk_b, Wv_w, Wv_b, Wo_w, Wo_b,
           **run_kwargs):
    global LAST_RESULT
    x = np.asarray(x, np.float32)
    mask = np.asarray(mask)
    Wq_w = np.asarray(Wq_w, np.float32)
    Wk_w = np.asarray(Wk_w, np.float32)
    Wv_w = np.asarray(Wv_w, np.float32)
    Wo_w = np.asarray(Wo_w, np.float32)

    nc = _get_program()

    xTs = [_bf(x[b].T) for b in range(B)]
    mbs = []
    for b in range(B):
        mrow = np.asarray(mask[b, 0, 0, :])
        bias = np.where(mrow == 0, np.float32(-50.0), np.float32(0.0))
        mbs.append(np.ascontiguousarray(bias.reshape(S // P, P).T.astype(np.float32)))

    in_maps = []
    for c in range(NCORES):
        b, g = c // 4, c % 4
        sl = slice(g * CL, (g + 1) * CL)
        in_maps.append({
            "xT": xTs[b],
            "wqT": _bf(Wq_w[sl, :].T),
            "wkT": _bf(Wk_w[sl, :].T),
            "wvT": _bf(Wv_w[sl, :].T),
            "bqp": np.ascontiguousarray(
                np.asarray(Wq_b, np.float32)[sl].reshape(NP, P).T),
            "mb": mbs[b],
            "woT": _bf(Wo_w[:, sl].T),
        })

    res = run_bass_kernel_spmd(nc, in_maps, core_ids=list(range(NCORES)),
                               **run_kwargs)
    LAST_RESULT = res

    # host-side unshard: sum the 4 row-parallel partials per batch and add
    # the folded constant bias (Wo @ Wv_b + Wo_b).
    obias = (Wo_w @ np.asarray(Wv_b, np.float32)
             + np.asarray(Wo_b, np.float32)).astype(np.float32)
    out = np.empty((B, S, D), np.float32)
    for b in range(B):
        acc = res.results[4 * b]["pout"].astype(np.float32)
        for g in range(1, 4):
            acc = acc + res.results[4 * b + g]["pout"].astype(np.float32)
        out[b] = acc + obias
    return out


# revision 7
# speedup vs baseline: 1.3692x; 1.0221x over previous
"""Multi-head self-attention on 8 Trainium2 NeuronCores.

Problem: B=2, S=2048, D=1024, H=16 heads (DK=64), fp32.

Sharding (8 cores): core c handles batch b = c//4 and head group g = c%4
(4 heads = 256 of the 1024 projection dims).  QKV are column-parallel,
Wo is row-parallel; the 4 partial outputs per batch are summed on the
host (cheap numpy add) together with a folded constant bias vector.

Device schedule (per core, identical SPMD program), bf16 operands:
  - The attention inner loop is jointly limited by ScalarE (exp of a
    [128,1024] score tile = ~1.34us) and the PE (scores + ctx + filler
    matmuls = ~1.1us/kt).  The emission order per key tile kt is
    scores(kt) -> exp(kt) -> ctx(kt-1) -> filler, so the in-order PE
    queue never blocks on the exp semaphore: exp(kt) input is complete
    one full iteration before ctx(kt) consumes it.
  - Q/K projections are packed per head PAIR (stationary M=128 instead
    of 64): qt2/kt2 hold two heads stacked on partitions [0:64]/[64:128]
    and the score matmuls slice a 64-partition base offset (PE tiling
    position (64,0) for odd heads).  Halves the projection instruction
    count and PE cycles vs per-head M=64.
  - Projection work (next pair's Q/K, first-half Wo) is drained from a
    unit queue 1-2 matmuls per kt iteration as TensorE filler inside the
    ACT-bound attention loop.
  - x is DMA'd in 4 column groups so the V projection (which reads all
    8 contraction chunks but only kt's 128 columns) starts at ~25% fill.
  - scores^T layout [kk, q] per (head, q-half): matmul -> PSUM[128,1024],
    exp(s/8 + mask_bias) fused on ScalarE -> bf16 P^T tiles,
    P^T @ V' (ones-column appended to V) accumulates context^T and the
    softmax denominators in one PSUM tile.
  - context is evicted unnormalized; the denominator row is reshaped to
    [128, 16] via a DRAM bounce so the iterative-divide reciprocal runs
    on 128 lanes, then broadcast back along partitions by DMA and
    applied with one tensor multiply.
  - Wo projection of normalized context^T -> partial out [2048, 1024]
    in bf16 (halves the output DMA; host accumulates in fp32).

PSUM budget (8 banks): score tiles [128,1024] x2 bufs (4) + context
accumulator [128,1024] (2) + projection tiles [128,512] x2 bufs (2).

Math notes (exactness):
  - K bias cancels in softmax (adds a per-query constant to scores).
  - V bias commutes: softmax(S) @ (V + 1 b_v^T) = softmax(S) @ V + b_v^T,
    so it is added on the host as Wo_w @ Wv_b (+ Wo_b) once per batch.
"""

import sys

for _p in ("/root/.axon_site", "/root/.axon_site/_ro/trn_rl_repo",
           "/root/.axon_site/_ro/pypackages", "/opt/trn_rl_repo"):
    if _p not in sys.path:
        sys.path.append(_p)

import ml_dtypes
import numpy as np

import concourse.bass as bass
import concourse.tile as tile
from concourse import bacc, mybir
from concourse.bass_utils import run_bass_kernel_spmd

B, S, D, H = 2, 2048, 1024, 16
DK = D // H          # 64 head dim
NCORES = 8
HL = H // 4          # 4 heads per core
NP = HL // 2         # 2 head pairs per core
CL = HL * DK         # 256 local context dims per core
P = 128
EC = D // P          # 8 contraction chunks
F32 = mybir.dt.float32
BF16 = mybir.dt.bfloat16
AF = mybir.ActivationFunctionType
BF = ml_dtypes.bfloat16

KT_TILES = S // P    # 16 key tiles
QW = 512             # matmul moving-dim chunk
SCW = 1024           # score-tile q width (one PSUM score tile)
NQH = S // SCW       # 2 q-halves per head
XG = 2               # x DMA column groups (2KB/partition lines)

LAST_RESULT = None   # BassKernelResults of the most recent run (for test.py)


def build_program():
    nc = bacc.Bacc("TRN2", target_bir_lowering=False, debug=False,
                   num_devices=NCORES)
    xT = nc.dram_tensor("xT", [D, S], BF16, kind="ExternalInput")
    wqT = nc.dram_tensor("wqT", [D, CL], BF16, kind="ExternalInput")
    wkT = nc.dram_tensor("wkT", [D, CL], BF16, kind="ExternalInput")
    wvT = nc.dram_tensor("wvT", [D, CL], BF16, kind="ExternalInput")
    bqp = nc.dram_tensor("bqp", [P, NP], F32, kind="ExternalInput")
    mb = nc.dram_tensor("mb", [P, KT_TILES], F32, kind="ExternalInput")
    woT = nc.dram_tensor("woT", [CL, D], BF16, kind="ExternalInput")
    pout = nc.dram_tensor("pout", [S, D], BF16, kind="ExternalOutput")

    with tile.TileContext(nc) as tc:
        with (
            tc.tile_pool(name="consts", bufs=1) as consts,
            tc.tile_pool(name="work", bufs=1) as work,
            tc.tile_pool(name="psum", bufs=1, space="PSUM") as psum,
            tc.tile_pool(name="dramp", bufs=2, space="DRAM") as dramp,
        ):
            # persistent SBUF tensors
            xt_sb = consts.tile([P, EC, S], BF16)
            wq_sb = consts.tile([P, EC, CL], BF16)
            wk_sb = consts.tile([P, EC, CL], BF16)
            wv_sb = consts.tile([P, EC, CL], BF16)
            v_sb = consts.tile([P, KT_TILES, HL, DK + 1], BF16)  # V + ones col
            ctxn = consts.tile([P, 2, S], BF16)                  # normalized ctx^T
            bq_sb = consts.tile([P, NP], F32)
            mb_sb = consts.tile([P, KT_TILES], F32)
            wo_sb = consts.tile([P, 2, D], BF16)

            # DMA order: V projection consumes wv + x column group g for key
            # tiles 8g..8g+7, so wv and group 0 go first; weights for the
            # projections that follow stream in between the x groups.
            xr = xT.rearrange("(j p) q -> p j q", p=P)
            GW = S // XG
            nc.sync.dma_start(out=wv_sb, in_=wvT.rearrange("(j p) c -> p j c", p=P))
            for g in range(XG):
                for e in range(EC):
                    nc.sync.dma_start(
                        out=xt_sb[:, e, g * GW:(g + 1) * GW],
                        in_=xr[:, e, g * GW:(g + 1) * GW])
                if g == 0:
                    nc.sync.dma_start(
                        out=wk_sb, in_=wkT.rearrange("(j p) c -> p j c", p=P))
                    nc.sync.dma_start(
                        out=wq_sb, in_=wqT.rearrange("(j p) c -> p j c", p=P))
            nc.sync.dma_start(out=bq_sb, in_=bqp[:, :])
            nc.sync.dma_start(out=mb_sb, in_=mb[:, :])
            nc.sync.dma_start(out=wo_sb, in_=woT.rearrange("(j p) c -> p j c", p=P))
            nc.vector.memset(v_sb[:, :, :, DK:DK + 1], 1.0)

            # ---- V projection (emitted interleaved with K/Q below) ----
            def v_proj(kt0, kt1):
                for kt in range(kt0, kt1):
                    ps = psum.tile([P, QW], F32, tag="pj", bufs=2, name=f"pv{kt}")
                    for e in range(EC):
                        nc.tensor.matmul(
                            ps[:, 0:CL],
                            lhsT=xt_sb[:, e, kt * P:(kt + 1) * P],
                            rhs=wv_sb[:, e, :],
                            start=(e == 0), stop=(e == EC - 1))
                    nc.vector.tensor_copy(
                        out=v_sb[:, kt, :, 0:DK],
                        in_=ps[:, 0:CL].rearrange("p (h d) -> p h d", h=HL))

            # ---- Q/K projection unit emitters (packed per head pair) ----
            # Each unit = one accumulation matmul; the 8th unit of a group
            # also evicts the PSUM tile into qt2/kt2.  Units are drained
            # 1-2 per attention iteration as TensorE filler.
            def qk_units(pair, qt2, kt2, qcs_q, qcs_k):
                units = []
                plo, phi = pair * P, (pair + 1) * P

                def emit(w_sb, o_t, is_q, qc):
                    ps_box = {}

                    def unit(e, w_sb=w_sb, o_t=o_t, is_q=is_q, qc=qc):
                        if e == 0:
                            ps_box[0] = psum.tile(
                                [P, QW], F32, tag="pj", bufs=2,
                                name=f"pqk{pair}_{int(is_q)}_{qc}")
                        ps = ps_box[0]
                        nc.tensor.matmul(
                            ps,
                            lhsT=w_sb[:, e, plo:phi],
                            rhs=xt_sb[:, e, qc * QW:(qc + 1) * QW],
                            start=(e == 0), stop=(e == EC - 1))
                        if e == EC - 1:
                            dst = o_t[:, qc * QW:(qc + 1) * QW]
                            if is_q:
                                nc.vector.tensor_scalar_add(
                                    out=dst, in0=ps,
                                    scalar1=bq_sb[:, pair:pair + 1])
                            else:
                                nc.vector.tensor_copy(out=dst, in_=ps)
                    return [lambda e=e: unit(e) for e in range(EC)]

                for qc in qcs_k:
                    units.extend(emit(wk_sb, kt2, False, qc))
                for qc in qcs_q:
                    units.extend(emit(wq_sb, qt2, True, qc))
                return units

            def alloc_qk(pair):
                qt2 = work.tile([P, S], BF16, tag="qt", bufs=2, name=f"qt{pair}")
                kt2 = work.tile([P, S], BF16, tag="kt", bufs=2, name=f"kt{pair}")
                return qt2, kt2

            # Pre-phase: interleave V tiles with pair-0 K/Q so PE work lines
            # up with x column-group arrival (V/K/Q chunks touching columns
            # 0:1024 need only group 0).  Q qc 2,3 become the first filler
            # units in head 0's attention.
            cur_qk = alloc_qk(0)
            v_proj(0, 8)
            for u in qk_units(0, *cur_qk, qcs_q=(), qcs_k=(0,)):
                u()
            for u in qk_units(0, *cur_qk, qcs_q=(0,), qcs_k=(1,)):
                u()
            v_proj(8, 12)
            for u in qk_units(0, *cur_qk, qcs_q=(1,), qcs_k=()):
                u()
            v_proj(12, 16)
            for u in qk_units(0, *cur_qk, qcs_q=(), qcs_k=(2, 3)):
                u()
            filler = list(qk_units(0, *cur_qk, qcs_q=(2, 3), qcs_k=()))

            # ---- Wo unit emitters (4 matmuls + eviction + DMA per tile) ----
            po_ctr = [0]

            def wo_units(t0, t1, act_evict):
                units = []
                for t in range(t0, t1):
                    box = {}

                    def unit(step, t=t, box=box):
                        if step == 0:
                            box["po"] = work.tile([P, D], BF16, tag="po",
                                                  bufs=3, name=f"po{t}")
                        dc, cb = divmod(step, 2)
                        ps_name = f"pw{t}_{dc}"
                        if cb == 0:
                            box[dc] = psum.tile([P, QW], F32, tag="pj",
                                                bufs=2, name=ps_name)
                        nc.tensor.matmul(
                            box[dc],
                            lhsT=ctxn[:, cb, t * P:(t + 1) * P],
                            rhs=wo_sb[:, cb, dc * QW:(dc + 1) * QW],
                            start=(cb == 0), stop=(cb == 1))
                        if cb == 1:
                            dst = box["po"][:, dc * QW:(dc + 1) * QW]
                            if act_evict and po_ctr[0] % 2 == 0:
                                nc.scalar.copy(out=dst, in_=box[dc])
                            else:
                                nc.vector.tensor_copy(out=dst, in_=box[dc])
                            po_ctr[0] += 1
                            if dc == 1:
                                nc.sync.dma_start(
                                    out=pout[t * P:(t + 1) * P, :],
                                    in_=box["po"])
                    units.extend([lambda s=s, u=unit: u(s) for s in range(4)])
                return units

            scale = 1.0 / float(np.sqrt(DK))
            for h in range(HL):
                pair, hh = divmod(h, 2)
                off = hh * DK
                qt2, kt2 = cur_qk
                if h == 1:
                    # next pair's projections become filler for the rest of
                    # pair 0's attention
                    nxt_qk = alloc_qk(1)
                    filler.extend(qk_units(1, *nxt_qk, qcs_q=(0, 1, 2, 3),
                                           qcs_k=(0, 1, 2, 3)))
                hb, hr = h // 2, (h % 2) * DK
                for qh in range(NQH):
                    q0 = qh * SCW
                    last = (h == HL - 1) and (qh == NQH - 1)
                    ctx_ps = psum.tile([P, SCW], F32, tag="ctx", bufs=1,
                                       name=f"ctx{h}_{qh}")
                    prev_pt = None
                    for kt in range(KT_TILES):
                        sc_ps = psum.tile([P, SCW], F32, tag="sc", bufs=2,
                                          name=f"sc{h}_{qh}_{kt}")
                        for c in range(SCW // QW):
                            nc.tensor.matmul(
                                sc_ps[:, c * QW:(c + 1) * QW],
                                lhsT=kt2[off:off + DK, kt * P:(kt + 1) * P],
                                rhs=qt2[off:off + DK,
                                        q0 + c * QW:q0 + (c + 1) * QW],
                                start=True, stop=True)
                        pt = work.tile([P, SCW], BF16, tag="pt", bufs=3,
                                       name=f"pt{h}_{qh}_{kt}")
                        nc.scalar.activation(out=pt, in_=sc_ps, func=AF.Exp,
                                             bias=mb_sb[:, kt:kt + 1],
                                             scale=scale)
                        if prev_pt is not None:
                            pkt, ppt = prev_pt
                            for c in range(SCW // QW):
                                nc.tensor.matmul(
                                    ctx_ps[0:DK + 1, c * QW:(c + 1) * QW],
                                    lhsT=v_sb[:, pkt, h, :],
                                    rhs=ppt[:, c * QW:(c + 1) * QW],
                                    start=(pkt == 0), stop=False)
                        prev_pt = (kt, pt)
                        # TensorE filler: drain 1-2 projection units
                        nticks = 2 if kt < 8 else 1
                        if last and kt == 4:
                            # first-half Wo becomes filler once head 3's
                            # first-half context is normalized
                            filler.extend(wo_units(0, S // P // 2, False))
                        for _ in range(nticks):
                            if filler:
                                filler.pop(0)()
                    pkt, ppt = prev_pt
                    for c in range(SCW // QW):
                        nc.tensor.matmul(
                            ctx_ps[0:DK + 1, c * QW:(c + 1) * QW],
                            lhsT=v_sb[:, pkt, h, :],
                            rhs=ppt[:, c * QW:(c + 1) * QW],
                            start=False, stop=True)
                    # evict unnormalized context + denominator row, then
                    # normalize this q-half (the chain overlaps the next
                    # q-half / head attention; DMA bounce reshapes the
                    # denominator so the reciprocal runs on 128 lanes)
                    ctxu = work.tile([DK, SCW], F32, tag="ctxu", bufs=3,
                                     name=f"ctxu{h}_{qh}")
                    nc.vector.tensor_copy(out=ctxu, in_=ctx_ps[0:DK, :])
                    den = work.tile([1, SCW], F32, tag="den", bufs=3,
                                    name=f"den{h}_{qh}")
                    nc.vector.tensor_copy(out=den, in_=ctx_ps[DK:DK + 1, :])
                    # the last q-half's chain is latency-critical: route its
                    # DMAs through the ACT hardware queue (idle once the exps
                    # are done) so they don't sit behind the pout DMAs.
                    dma = nc.scalar.dma_start if last else nc.sync.dma_start
                    dd = dramp.tile([1, SCW], F32, tag="dd", name=f"dd{h}{qh}")
                    dma(out=dd, in_=den)
                    den2 = work.tile([P, SCW // P], F32, tag="den2", bufs=3,
                                     name=f"den2{h}_{qh}")
                    dma(out=den2, in_=dd.rearrange("o (p f) -> (o p) f", p=P))
                    den2r = work.tile([P, SCW // P], F32, tag="den2r", bufs=3,
                                      name=f"den2r{h}_{qh}")
                    nc.vector.reciprocal(out=den2r, in_=den2)
                    dr = dramp.tile([1, SCW], F32, tag="dr", name=f"dr{h}{qh}")
                    dma(out=dr.rearrange("o (p f) -> (o p) f", p=P), in_=den2r)
                    rb = work.tile([DK, SCW], F32, tag="rb", bufs=3,
                                   name=f"rb{h}_{qh}")
                    dma(out=rb, in_=dr.to_broadcast([DK, SCW]))
                    nc.vector.tensor_mul(out=ctxn[hr:hr + DK, hb, q0:q0 + SCW],
                                         in0=ctxu, in1=rb)
                if pair == 0 and hh == 1:
                    cur_qk = nxt_qk

            # ---- remaining output projection (second q-half + leftovers) ----
            for u in filler:
                u()
            for u in wo_units(S // P // 2, S // P, True):
                u()

    nc.compile()
    return nc


_PROGRAM = None


def _get_program():
    global _PROGRAM
    if _PROGRAM is None:
        _PROGRAM = build_program()
    return _PROGRAM


def _bf(a):
    return np.ascontiguousarray(np.asarray(a, np.float32)).astype(BF)


def kernel(x, mask, Wq_w, Wq_b, Wk_w, Wk_b, Wv_w, Wv_b, Wo_w, Wo_b,
           **run_kwargs):
    global LAST_RESULT
    x = np.asarray(x, np.float32)
    mask = np.asarray(mask)
    Wq_w = np.asarray(Wq_w, np.float32)
    Wk_w = np.asarray(Wk_w, np.float32)
    Wv_w = np.asarray(Wv_w, np.float32)
    Wo_w = np.asarray(Wo_w, np.float32)

    nc = _get_program()

    xTs = [_bf(x[b].T) for b in range(B)]
    mbs = []
    for b in range(B):
        mrow = np.asarray(mask[b, 0, 0, :])
        bias = np.where(mrow == 0, np.float32(-50.0), np.float32(0.0))
        mbs.append(np.ascontiguousarray(bias.reshape(S // P, P).T.astype(np.float32)))

    in_maps = []
    for c in range(NCORES):
        b, g = c // 4, c % 4
        sl = slice(g * CL, (g + 1) * CL)
        in_maps.append({
            "xT": xTs[b],
            "wqT": _bf(Wq_w[sl, :].T),
            "wkT": _bf(Wk_w[sl, :].T),
            "wvT": _bf(Wv_w[sl, :].T),
            "bqp": np.ascontiguousarray(
                np.asarray(Wq_b, np.float32)[sl].reshape(NP, P).T),
            "mb": mbs[b],
            "woT": _bf(Wo_w[:, sl].T),
        })

    res = run_bass_kernel_spmd(nc, in_maps, core_ids=list(range(NCORES)),
                               **run_kwargs)
    LAST_RESULT = res

    # host-side unshard: sum the 4 row-parallel partials per batch and add
    # the folded constant bias (Wo @ Wv_b + Wo_b).
    obias = (Wo_w @ np.asarray(Wv_b, np.float32)
             + np.asarray(Wo_b, np.float32)).astype(np.float32)
    out = np.empty((B, S, D), np.float32)
    for b in range(B):
        acc = res.results[4 * b]["pout"].astype(np.float32)
        for g in range(1, 4):
            acc = acc + res.results[4 * b + g]["pout"].astype(np.float32)
        out[b] = acc + obias
    return out
